# revision 7
# baseline (speedup 1.0000x reference)
"""Trainium2 Bass kernel for nn_BIGRU: BatchNorm + 3-layer bidirectional GRU
(both directions run forward in time) + final FC.

Sharding: pure data-parallel over batch N=256 across 8 cores (32 rows each).
Each core computes both GRU directions for its batch shard; no cross-core
communication. BN batch stats are computed on the host and folded into the
layer-0 projection weights (exact algebra, negligible host flops).

Device layout convention: [features/gates on partitions, tokens/batch on free].
  x_l     DRAM [128, 16, 2048]   x_l[p, d*8+c, t*32+n] = feature (c*128+p) of dir d
  gi      DRAM [128, 24, 2048]   gi[p, j, t*32+n] = gate channel (j*128+p)
  w_ih^T  DRAM [128, KT, 24, 128]
  w_hh^T  DRAM [128, 8, 3072]
Gate order (PyTorch GRU): r = gates[0:1024], z = [1024:2048], n = [2048:3072].
Step math (biases pre-folded into gi except b_hh_n = gamma):
  r = sigmoid(gi_r + (h W^T)_r)        # gi_r includes b_ih_r + b_hh_r
  z = sigmoid(gi_z + (h W^T)_z)
  n = tanh(gi_n + r * ((h W^T)_n + gamma))   # gi_n includes b_ih_n
  h' = z*(h - n) + n
"""
import sys

for _p in ("/opt/trn_rl_repo", "/root/.axon_site/_ro/trn_rl_repo"):
    if _p not in sys.path:
        sys.path.append(_p)

import numpy as np

import concourse.bass as bass
import concourse.mybir as mybir
import concourse.tile as tile
from concourse.bass import ds
from concourse.bass_utils import run_bass_kernel_spmd

FP = mybir.dt.float32
AF = mybir.ActivationFunctionType
OP = mybir.AluOpType

N_CORES = 8
N, T, IN_SIZE = 256, 64, 150
H = 1024
NC_B = N // N_CORES          # 32 batch rows per core
NT = T * NC_B                # 2048 tokens per core
G3 = 3 * H                   # 3072 gate channels
NJ = G3 // 128               # 24 gate tiles
NI = H // 128                # 8 hidden tiles
EPS = 1e-5
SCAN_UNROLL = 4


def _split_multi_waits(nc, max_waits=1):
    """walrus in this container rejects >1 sync wait per instruction; hoist
    extra waits onto single-wait NoOps on the same engine (queues are
    in-order, so this is semantically equivalent)."""
    ctr = 0
    n_split = 0
    for fn in nc.m.functions:
        for blk in fn.blocks:
            insts = list(blk.instructions)
            out = []
            changed = False
            for inst in insts:
                si = inst.sync_info
                waits = list(si.on_wait) if si else []
                if len(waits) > max_waits:
                    keep = waits[-max_waits:]
                    for w in waits[:-max_waits]:
                        ctr += 1
                        out.append(
                            mybir.InstNoOp(
                                name=f"waitnop_{ctr}",
                                engine=inst.engine,
                                ins=[],
                                outs=[],
                                sync_info=mybir.SyncInfo(on_wait=[w], on_update=[]),
                            )
                        )
                        n_split += 1
                    inst.sync_info = mybir.SyncInfo(
                        on_wait=keep, on_update=list(si.on_update)
                    )
                    changed = True
                out.append(inst)
            if changed:
                blk.instructions.clear()
                for i in out:
                    blk.instructions.append(i)
    return n_split


def _emit_proj(nc, tc, sb, psum, x_src, kt, w_d, beta_t, gi_d):
    """gi[:, g, :] (+= beta) = sum_k w[k,g].T @ x[k] for all 2048 tokens.

    x_src: SBUF tile [128, kt, 2048]; w_d: DRAM [128, kt, 24, 128];
    beta_t: SBUF [128, 24]; gi_d: DRAM [128, 24, 2048].
    """

    def body(g):
        wt = sb.tile([128, kt, 128], FP, tag="projw")
        nc.sync.dma_start(wt[:], w_d[:, :, g, :])
        acc = psum.tile([128, 4, 512], FP, tag="projp")
        for c in range(4):
            for k in range(kt):
                nc.tensor.matmul(
                    acc[:, c, :],
                    wt[:, k, :],
                    x_src[:, k, ds(c * 512, 512)],
                    start=(k == 0),
                    stop=(k == kt - 1),
                )
        stage = sb.tile([128, 2048], FP, tag="projs")
        for c in range(4):
            nc.scalar.activation(
                stage[:, ds(c * 512, 512)],
                acc[:, c, :],
                AF.Identity,
                bias=beta_t[:, g : g + 1],
            )
        nc.sync.dma_start(gi_d[:, g, :], stage[:])

    for g in range(NJ):
        body(g)


def _emit_scan(nc, tc, wpool, sb, psum, whh_d, grep_d, gi_d, xo_d, d, hpool, htag):
    """64-step GRU scan for one direction. Returns the final-h SBUF tile.

    whh_d: DRAM [128, 8, 3072]; grep_d: DRAM [128, 8, 32] (gamma replicated
    over batch); gi_d: DRAM [128, 24, 2048]; xo_d: DRAM [128, 16, 2048]
    (output x of this layer; this scan writes dir-half d).
    """
    wh = wpool.tile([128, 8, 3072], FP, tag="whh")
    nc.sync.dma_start(wh[:], whh_d[:])
    grep = wpool.tile([128, 8, 32], FP, tag="grep")
    nc.sync.dma_start(grep[:], grep_d[:])
    hstate = hpool.tile([128, 8, 32], FP, tag=htag)
    nc.vector.memset(hstate[:], 0.0)

    U = SCAN_UNROLL

    def step_block(tcb):
        gb = sb.tile([128, 24, 32 * U], FP, tag="gib")
        nc.sync.dma_start(gb[:], gi_d[:, :, ds(tcb, 32 * U)])
        hs = sb.tile([128, 8, U, 32], FP, tag="hstage")
        for s in range(U):
            ssl = ds(s * 32, 32)
            pr = psum.tile([128, 8, 32], FP, tag="pr")
            pz = psum.tile([128, 8, 32], FP, tag="pz")
            pn = psum.tile([128, 8, 32], FP, tag="pn")
            for j in range(NJ):
                p_out = (pr, pz, pn)[j // 8]
                for i in range(NI):
                    nc.tensor.matmul(
                        p_out[:, j % 8, :],
                        wh[:, i, ds(j * 128, 128)],
                        hstate[:, i, :],
                        start=(i == 0),
                        stop=(i == NI - 1),
                    )
            ar = sb.tile([128, 8, 32], FP, tag="ar")
            nc.vector.tensor_tensor(ar[:], pr[:], gb[:, 0:8, ssl], op=OP.add)
            rr = sb.tile([128, 8, 32], FP, tag="rr")
            nc.scalar.activation(rr[:], ar[:], AF.Sigmoid)
            az = sb.tile([128, 8, 32], FP, tag="az")
            nc.vector.tensor_tensor(az[:], pz[:], gb[:, 8:16, ssl], op=OP.add)
            zz = sb.tile([128, 8, 32], FP, tag="zz")
            nc.scalar.activation(zz[:], az[:], AF.Sigmoid)
            t1 = sb.tile([128, 8, 32], FP, tag="t1")
            nc.vector.tensor_tensor(t1[:], pn[:], grep[:], op=OP.add)
            t2 = sb.tile([128, 8, 32], FP, tag="t2")
            nc.vector.tensor_tensor(t2[:], t1[:], rr[:], op=OP.mult)
            t3 = sb.tile([128, 8, 32], FP, tag="t3")
            nc.vector.tensor_tensor(t3[:], t2[:], gb[:, 16:24, ssl], op=OP.add)
            nn_ = sb.tile([128, 8, 32], FP, tag="nn")
            nc.scalar.activation(nn_[:], t3[:], AF.Tanh)
            dd = sb.tile([128, 8, 32], FP, tag="dd")
            nc.vector.tensor_tensor(dd[:], hstate[:], nn_[:], op=OP.subtract)
            ee = sb.tile([128, 8, 32], FP, tag="ee")
            nc.vector.tensor_tensor(ee[:], dd[:], zz[:], op=OP.mult)
            nc.vector.tensor_tensor(hstate[:], ee[:], nn_[:], op=OP.add)
            nc.scalar.activation(hs[:, :, s, :], hstate[:], AF.Copy)
        nc.sync.dma_start(xo_d[:, ds(d * 8, 8), ds(tcb, 32 * U)], hs[:])

    with tc.For_i(0, NT, 32 * U) as tcb:
        step_block(tcb)
    return hstate


_CACHE = {}


def _build():
    if "nc" in _CACHE:
        return _CACHE["nc"]
    nc = bass.Bass()

    xt_d = nc.dram_tensor("xt", [128, 2, NT], FP, kind="ExternalInput")
    w0_d = [
        nc.dram_tensor(f"w0_{d}", [128, 2, NJ, 128], FP, kind="ExternalInput")
        for d in range(2)
    ]
    wih_d = {
        (l, d): nc.dram_tensor(f"wih{l}_{d}", [128, 16, NJ, 128], FP, kind="ExternalInput")
        for l in (1, 2)
        for d in range(2)
    }
    whh_d = {
        (l, d): nc.dram_tensor(f"whh{l}_{d}", [128, 8, G3], FP, kind="ExternalInput")
        for l in range(3)
        for d in range(2)
    }
    beta_d = {
        (l, d): nc.dram_tensor(f"beta{l}_{d}", [128, NJ], FP, kind="ExternalInput")
        for l in range(3)
        for d in range(2)
    }
    grep_d = {
        (l, d): nc.dram_tensor(f"grep{l}_{d}", [128, 8, 32], FP, kind="ExternalInput")
        for l in range(3)
        for d in range(2)
    }
    fcw_d = nc.dram_tensor("fcw", [128, 16, 60], FP, kind="ExternalInput")
    fcb_d = nc.dram_tensor("fcb", [60, 1], FP, kind="ExternalInput")

    x1_d = nc.dram_tensor("x1", [128, 16, NT], FP)
    gi_d = {d: nc.dram_tensor(f"gi_{d}", [128, NJ, NT], FP) for d in range(2)}
    enc_d = nc.dram_tensor("enc", [128, 16, NT], FP, kind="ExternalOutput")
    log_d = nc.dram_tensor("logits", [60, NC_B], FP, kind="ExternalOutput")

    with tile.TileContext(nc) as tc:
        with tc.tile_pool(name="hst", bufs=1) as hpool:
            # betas (all layers/dirs) loaded once
            beta_t = {}
            for l in range(3):
                for d in range(2):
                    bt = hpool.tile([128, NJ], FP, tag=f"beta{l}{d}")
                    nc.sync.dma_start(bt[:], beta_d[(l, d)][:])
                    beta_t[(l, d)] = bt

            xbufs = {0: x1_d, 1: x1_d, 2: enc_d}  # output x of layer l
            hfin = {}
            for l in range(3):
                # ---- projection for both dirs ----
                with (
                    tc.tile_pool(name=f"px{l}", bufs=1) as px,
                    tc.tile_pool(name=f"pw{l}", bufs=3) as pw,
                    tc.tile_pool(name=f"pp{l}", bufs=2, space="PSUM") as pp,
                ):
                    if l == 0:
                        xr = px.tile([128, 2, NT], FP, tag="xr")
                        nc.sync.dma_start(xr[:], xt_d[:])
                        kt = 2
                        wsrc = {d: w0_d[d] for d in range(2)}
                    else:
                        xr = px.tile([128, 16, NT], FP, tag="xr")
                        nc.sync.dma_start(xr[:], xbufs[l - 1][:])
                        kt = 16
                        wsrc = {d: wih_d[(l, d)] for d in range(2)}
                    for d in range(2):
                        _emit_proj(nc, tc, pw, pp, xr, kt, wsrc[d],
                                   beta_t[(l, d)], gi_d[d])
                # ---- scans ----
                for d in range(2):
                    with (
                        tc.tile_pool(name=f"swh{l}{d}", bufs=1) as swh,
                        tc.tile_pool(name=f"sw{l}{d}", bufs=2) as sw,
                        tc.tile_pool(name=f"sp{l}{d}", bufs=2, space="PSUM") as sp,
                    ):
                        hfin[(l, d)] = _emit_scan(
                            nc, tc, swh, sw, sp, whh_d[(l, d)], grep_d[(l, d)],
                            gi_d[d], xbufs[l], d, hpool, f"h{l}{d}",
                        )

            # ---- FC: logits[60, 32] = fc_w @ h_final + fc_b ----
            with (
                tc.tile_pool(name="fcs", bufs=1) as fcs,
                tc.tile_pool(name="fcp", bufs=1, space="PSUM") as fcpp,
            ):
                fcw = fcs.tile([128, 16, 60], FP, tag="fcw")
                nc.sync.dma_start(fcw[:], fcw_d[:])
                fcb = fcs.tile([60, 1], FP, tag="fcb")
                nc.sync.dma_start(fcb[:], fcb_d[:])
                fp_ = fcpp.tile([60, NC_B], FP, tag="fcp")
                for d in range(2):
                    hf = hfin[(2, d)]
                    for i in range(NI):
                        k = d * 8 + i
                        nc.tensor.matmul(
                            fp_[:],
                            fcw[:, k, :],
                            hf[:, i, :],
                            start=(k == 0),
                            stop=(k == 15),
                        )
                fo = fcs.tile([60, NC_B], FP, tag="fco")
                nc.scalar.activation(fo[:], fp_[:], AF.Identity, bias=fcb[:])
                nc.sync.dma_start(log_d[:], fo[:])

    _split_multi_waits(nc)
    _CACHE["nc"] = nc
    return nc


def _pack_gates_T(w):
    """w: [G3, K] -> DRAM layout [128, K//128, 24, 128]: out[p,k,g,c] = w[g*128+c, k*128+p]."""
    K = w.shape[1]
    kt = (K + 127) // 128
    out = np.zeros((128, kt, NJ, 128), np.float32)
    wt = np.ascontiguousarray(w.T)  # [K, G3]
    for k in range(kt):
        rows = wt[k * 128 : (k + 1) * 128]  # [<=128, G3]
        r = rows.shape[0]
        out[:r, k] = rows.reshape(r, NJ, 128)
    return out


def _prep_inputs(x, bn_gamma, bn_beta, ws):
    """Build the 8 per-core input maps. ws[(l,d)] = (w_ih, w_hh, b_ih, b_hh),
    plus fc_w, fc_b in ws['fc']."""
    x = np.asarray(x, np.float32)
    mean = x.mean(axis=(0, 1))
    var = x.var(axis=(0, 1))
    a = np.asarray(bn_gamma, np.float32) / np.sqrt(var + EPS)
    b = np.asarray(bn_beta, np.float32) - mean * a

    shared = {}
    for d in range(2):
        w_ih, w_hh, b_ih, b_hh = ws[(0, d)]
        w0 = np.asarray(w_ih, np.float32) * a[None, :]       # [3072, 150]
        beta = np.asarray(w_ih, np.float32) @ b + np.asarray(b_ih, np.float32)
        beta[: 2 * H] += np.asarray(b_hh, np.float32)[: 2 * H]
        gamma = np.asarray(b_hh, np.float32)[2 * H :]
        shared[f"w0_{d}"] = _pack_gates_T(w0)
        shared[f"beta0_{d}"] = beta.reshape(NJ, 128).T.copy()
        shared[f"grep0_{d}"] = np.repeat(
            gamma.reshape(8, 128).T[:, :, None], NC_B, axis=2
        ).astype(np.float32)
        shared[f"whh0_{d}"] = (
            np.asarray(w_hh, np.float32).T.reshape(8, 128, G3).transpose(1, 0, 2).copy()
        )
    for l in (1, 2):
        for d in range(2):
            w_ih, w_hh, b_ih, b_hh = ws[(l, d)]
            beta = np.asarray(b_ih, np.float32).copy()
            beta[: 2 * H] += np.asarray(b_hh, np.float32)[: 2 * H]
            gamma = np.asarray(b_hh, np.float32)[2 * H :]
            shared[f"wih{l}_{d}"] = _pack_gates_T(np.asarray(w_ih, np.float32))
            shared[f"beta{l}_{d}"] = beta.reshape(NJ, 128).T.copy()
            shared[f"grep{l}_{d}"] = np.repeat(
                gamma.reshape(8, 128).T[:, :, None], NC_B, axis=2
            ).astype(np.float32)
            shared[f"whh{l}_{d}"] = (
                np.asarray(w_hh, np.float32).T.reshape(8, 128, G3)
                .transpose(1, 0, 2)
                .copy()
            )
    fc_w, fc_b = ws["fc"]
    shared["fcw"] = (
        np.asarray(fc_w, np.float32).T.reshape(16, 128, 60).transpose(1, 0, 2).copy()
    )
    shared["fcb"] = np.asarray(fc_b, np.float32).reshape(60, 1).copy()

    in_maps = []
    for c in range(N_CORES):
        xs = x[c * NC_B : (c + 1) * NC_B]            # [32, 64, 150]
        xtp = np.zeros((128, 2, NT), np.float32)
        xf = xs.transpose(2, 1, 0).reshape(IN_SIZE, NT)  # [150, t*32+n]
        xtp[:, 0, :] = xf[:128]
        xtp[: IN_SIZE - 128, 1, :] = xf[128:]
        m = dict(shared)
        m["xt"] = xtp
        in_maps.append(m)
    return in_maps


def kernel(x, bn_gamma, bn_beta,
           w_ih_0, w_hh_0, b_ih_0, b_hh_0,
           w_ih_1, w_hh_1, b_ih_1, b_hh_1,
           w_ih_2, w_hh_2, b_ih_2, b_hh_2,
           fc_w, fc_b):
    ws = {}
    for l, (wi, wh, bi, bh) in enumerate(
        [(w_ih_0, w_hh_0, b_ih_0, b_hh_0),
         (w_ih_1, w_hh_1, b_ih_1, b_hh_1),
         (w_ih_2, w_hh_2, b_ih_2, b_hh_2)]
    ):
        for d in range(2):
            ws[(l, d)] = (
                np.asarray(wi)[d], np.asarray(wh)[d],
                np.asarray(bi)[d], np.asarray(bh)[d],
            )
    ws["fc"] = (np.asarray(fc_w), np.asarray(fc_b))

    nc = _build()
    in_maps = _prep_inputs(np.asarray(x), bn_gamma, bn_beta, ws)
    res = run_bass_kernel_spmd(nc, in_maps, list(range(N_CORES)))

    enc_parts = []
    log_parts = []
    for c in range(N_CORES):
        enc = res.results[c]["enc"]        # [128, 16, 2048]
        # enc[p, d*8+c, t*32+n] -> [32, 64, 2048]
        e = enc.reshape(128, 16, T, NC_B).transpose(3, 2, 1, 0).reshape(NC_B, T, 2048)
        enc_parts.append(e)
        log_parts.append(res.results[c]["logits"].T)  # [32, 60]
    encoder_hidden = np.concatenate(enc_parts, axis=0)
    out = np.concatenate(log_parts, axis=0)
    return out, encoder_hidden


# revision 43
# speedup vs baseline: 4797.4651x; 4797.4651x over previous
"""Trainium2 Bass kernel for nn_BIGRU: BatchNorm + 3-layer bidirectional GRU
(both directions run forward in time) + final FC.

Sharding: pure data-parallel over batch N=256 across 8 cores (32 rows each).
Each core computes both GRU directions for its batch shard; no cross-core
communication. BN batch stats are computed on the host and folded into the
layer-0 projection weights (exact algebra, negligible host flops).

Device layout convention: [features/gates on partitions, tokens/batch on free].
  x_l     DRAM [128, 16, 2048]   x_l[p, d*8+c, t*32+n] = feature (c*128+p) of dir d
  gi      DRAM [128, 24, 2048]   gi[p, j, t*32+n] = gate channel (j*128+p)
  w_ih^T  DRAM [128, KT, 24, 128]
  w_hh^T  DRAM [128, 8, 3072]
Gate order (PyTorch GRU): r = gates[0:1024], z = [1024:2048], n = [2048:3072].
Step math (biases pre-folded into gi except b_hh_n = gamma):
  r = sigmoid(gi_r + (h W^T)_r)        # gi_r includes b_ih_r + b_hh_r
  z = sigmoid(gi_z + (h W^T)_z)
  n = tanh(gi_n + r * ((h W^T)_n + gamma))   # gi_n includes b_ih_n
  h' = z*(h - n) + n
"""
import sys

for _p in ("/opt/trn_rl_repo", "/root/.axon_site/_ro/trn_rl_repo"):
    if _p not in sys.path:
        sys.path.append(_p)

import numpy as np

import ml_dtypes

import concourse.bass as bass
import concourse.mybir as mybir
import concourse.tile as tile
from concourse.bass import ds
from concourse.bass_utils import run_bass_kernel_spmd
from concourse.engine_type import EngineType

FP = mybir.dt.float32
F32R = mybir.dt.float32r
BF = mybir.dt.bfloat16
BF_NP = ml_dtypes.bfloat16
AF = mybir.ActivationFunctionType
OP = mybir.AluOpType

N_CORES = 8
N, T, IN_SIZE = 256, 64, 150
H = 1024
NC_B = N // N_CORES          # 32 batch rows per core
NT = T * NC_B                # 2048 tokens per core
G3 = 3 * H                   # 3072 gate channels
NJ = G3 // 128               # 24 gate tiles
NI = H // 128                # 8 hidden tiles
EPS = 1e-5
SCAN_UNROLL = 8
SCAN_BF16 = True   # bf16 recurrent weights + state (FWL halves weight-load time)
PROJ_BF16 = True   # bf16 input projections (1 cyc/row vs fp32's 4, FWL)
TIME_REPS = 1      # >1: repeat each compute phase for timing (wrecks numerics)
SCAN_W_FP8 = False  # fp8-e4m3 recurrent weights (4x FWL) — ~1.3e-2 rel err
FP8_SCALE = 8192.0  # power of two; keeps w_hh*scale in e4m3 normal range


def _rep_loop(tc, name):
    import contextlib
    if TIME_REPS == 1:
        return contextlib.nullcontext()
    return tc.For_i(0, TIME_REPS, 1, name=name)


def _split_multi_waits(nc, max_waits=1):
    """walrus in this container rejects >1 sync wait per instruction; hoist
    extra waits onto single-wait NoOps on the same engine (queues are
    in-order, so this is semantically equivalent)."""
    ctr = 0
    n_split = 0
    for fn in nc.m.functions:
        for blk in fn.blocks:
            insts = list(blk.instructions)
            out = []
            changed = False
            for inst in insts:
                si = inst.sync_info
                waits = list(si.on_wait) if si else []
                if len(waits) > max_waits:
                    keep = waits[-max_waits:]
                    for w in waits[:-max_waits]:
                        ctr += 1
                        out.append(
                            mybir.InstNoOp(
                                name=f"waitnop_{ctr}",
                                engine=inst.engine,
                                ins=[],
                                outs=[],
                                sync_info=mybir.SyncInfo(on_wait=[w], on_update=[]),
                            )
                        )
                        n_split += 1
                    inst.sync_info = mybir.SyncInfo(
                        on_wait=keep, on_update=list(si.on_update)
                    )
                    changed = True
                out.append(inst)
            if changed:
                blk.instructions.clear()
                for i in out:
                    blk.instructions.append(i)
    return n_split


def _emit_proj(nc, tc, sb, psum, x_src, kt, w_d, beta_t, gi_d):
    """gi[:, g, :] (+= beta) = sum_k w[k,g].T @ x[k] for all 2048 tokens.

    x_src: SBUF tile [128, kt, 2048]; w_d: DRAM [128, kt, 24, 128];
    beta_t: SBUF [128, 24]; gi_d: DRAM [128, 24, 2048].
    """

    def body(g):
        wt = sb.tile([128, kt, 128], BF if PROJ_BF16 else FP, tag="projw")
        nc.sync.dma_start(wt[:], w_d[:, :, g, :])
        acc = psum.tile([128, 4, 512], FP, tag="projp")
        for k in range(kt):
            for c in range(4):
                nc.tensor.matmul(
                    acc[:, c, :],
                    wt[:, k, :],
                    x_src[:, k, ds(c * 512, 512)],
                    start=(k == 0),
                    stop=(k == kt - 1),
                )
        stage = sb.tile([128, 2048], FP, tag="projs")
        for c in range(4):
            nc.scalar.activation(
                stage[:, ds(c * 512, 512)],
                acc[:, c, :],
                AF.Identity,
                bias=beta_t[:, g : g + 1],
            )
        nc.sync.dma_start(gi_d[:, g, :], stage[:])

    for g in range(NJ):
        body(g)


def _emit_scan_pair(nc, tc, wpool, sb, psum, whh_ds, grep_ds, gi_ds, xo_d,
                    hpool, l, last=False):
    """64-step GRU scans for BOTH directions, interleaved step by step so one
    direction's gate math (DVE/ACT) hides under the other's matmul phase.
    Returns (h_final_fwd, h_final_bwd) SBUF tiles.
    """
    sdt = BF if SCAN_BF16 else FP
    wdt = mybir.dt.float8e4 if SCAN_W_FP8 else sdt
    wh, grep, hstate = {}, {}, {}
    for d in range(2):
        wh[d] = wpool.tile([128, 8, 3072], wdt, tag=f"whh{d}", name=f"whh{l}{d}")
        nc.sync.dma_start(wh[d][:], whh_ds[d][:])
        grep[d] = wpool.tile([128, 8, 32], FP, tag=f"grep{d}", name=f"grep{l}{d}")
        nc.sync.dma_start(grep[d][:], grep_ds[d][:])
        hstate[d] = hpool.tile([128, 8, 32], sdt, tag=f"h{l}{d}", name=f"hst{l}{d}")
        nc.vector.memset(hstate[d][:], 0.0)

    U = SCAN_UNROLL
    UH = U // 2
    hs_dt = FP if (last or not PROJ_BF16) else BF

    def emit_mm(d, s, ps):
        pr, pz, pn = ps
        for j in range(NJ):
            p_out = (pr, pz, pn)[j // 8]
            for i in range(NI):
                nc.tensor.matmul(
                    p_out[:, j % 8, :],
                    wh[d][:, i, ds(j * 128, 128)],
                    hstate[d][:, i, :],
                    start=(i == 0),
                    stop=(i == NI - 1),
                )

    def emit_gates(d, s, ps, gb, ssl, hs):
        pr, pz, pn = ps
        inv = 1.0 / FP8_SCALE
        ar = sb.tile([128, 8, 32], FP, tag="ar")
        az = sb.tile([128, 8, 32], FP, tag="az")
        t1 = sb.tile([128, 8, 32], FP, tag="t1")
        if SCAN_W_FP8:
            nc.vector.scalar_tensor_tensor(
                ar[:], pr[:], inv, gb[:, 0:8, ssl], op0=OP.mult, op1=OP.add)
        else:
            nc.vector.tensor_tensor(ar[:], pr[:], gb[:, 0:8, ssl], op=OP.add)
        rr = sb.tile([128, 8, 32], FP, tag="rr")
        nc.scalar.activation(rr[:], ar[:], AF.Sigmoid)
        if SCAN_W_FP8:
            nc.vector.scalar_tensor_tensor(
                az[:], pz[:], inv, gb[:, 8:16, ssl], op0=OP.mult, op1=OP.add)
        else:
            nc.vector.tensor_tensor(az[:], pz[:], gb[:, 8:16, ssl], op=OP.add)
        zz = sb.tile([128, 8, 32], FP, tag="zz")
        nc.scalar.activation(zz[:], az[:], AF.Sigmoid)
        if SCAN_W_FP8:
            nc.vector.scalar_tensor_tensor(
                t1[:], pn[:], inv, grep[d][:], op0=OP.mult, op1=OP.add)
        else:
            nc.vector.tensor_tensor(t1[:], pn[:], grep[d][:], op=OP.add)
        t2 = sb.tile([128, 8, 32], FP, tag="t2")
        nc.vector.tensor_tensor(t2[:], t1[:], rr[:], op=OP.mult)
        t3 = sb.tile([128, 8, 32], FP, tag="t3")
        nc.vector.tensor_tensor(t3[:], t2[:], gb[:, 16:24, ssl], op=OP.add)
        nn_ = sb.tile([128, 8, 32], FP, tag="nn")
        nc.scalar.activation(nn_[:], t3[:], AF.Tanh)
        dd = sb.tile([128, 8, 32], FP, tag="dd")
        nc.vector.tensor_tensor(dd[:], hstate[d][:], nn_[:], op=OP.subtract)
        ee = sb.tile([128, 8, 32], FP, tag="ee")
        nc.vector.tensor_tensor(ee[:], dd[:], zz[:], op=OP.mult)
        nc.vector.tensor_tensor(hs[:, :, s, :], ee[:], nn_[:], op=OP.add)
        nc.scalar.activation(hstate[d][:], hs[:, :, s, :], AF.Copy)

    def step_block(tcb):
        gb, hs = {}, {}
        for d in range(2):
            g0 = wpool.tile([128, 24, 32 * UH], FP, tag=f"gib0{d}")
            nc.sync.dma_start(g0[:], gi_ds[d][:, :, ds(tcb, 32 * UH)])
            g1 = wpool.tile([128, 24, 32 * UH], FP, tag=f"gib1{d}")
            nc.sync.dma_start(g1[:], gi_ds[d][:, :, ds(tcb + 32 * UH, 32 * UH)])
            gb[d] = (g0, g1)
            hs[d] = sb.tile([128, 8, U, 32], hs_dt, tag=f"hstage{d}", name=f"hstage{d}")
        ps = {}
        for s in range(U):
            ssl = ds((s % UH) * 32, 32)
            for d in range(2):
                ps[d] = tuple(
                    psum.tile([128, 8, 32], FP, tag=f"p{gate}{d}",
                              name=f"p{gate}{d}")
                    for gate in "rzn"
                )
                emit_mm(d, s, ps[d])
                emit_gates(d, s, ps[d], gb[d][0 if s < UH else 1], ssl, hs[d])
        for d in range(2):
            nc.sync.dma_start(xo_d[:, ds(d * 8, 8), ds(tcb, 32 * U)], hs[d][:])

    with _rep_loop(tc, f"rep_s{l}"):
        with tc.For_i(0, NT, 32 * U, hint_engines=(EngineType.PE,)) as tcb:
            step_block(tcb)
    return hstate[0], hstate[1]


_CACHE = {}


def _build():
    if "nc" in _CACHE:
        return _CACHE["nc"]
    nc = bass.Bass()

    PDT = BF if PROJ_BF16 else FP
    xt_d = nc.dram_tensor("xt", [128, 2, NT], PDT, kind="ExternalInput")
    w0_d = [
        nc.dram_tensor(f"w0_{d}", [128, 2, NJ, 128], PDT, kind="ExternalInput")
        for d in range(2)
    ]
    wih_d = {
        (l, d): nc.dram_tensor(f"wih{l}_{d}", [128, 16, NJ, 128], PDT, kind="ExternalInput")
        for l in (1, 2)
        for d in range(2)
    }
    whh_d = {
        (l, d): nc.dram_tensor(
            f"whh{l}_{d}", [128, 8, G3],
            mybir.dt.float8e4 if SCAN_W_FP8 else (BF if SCAN_BF16 else FP),
            kind="ExternalInput")
        for l in range(3)
        for d in range(2)
    }
    beta_d = {
        (l, d): nc.dram_tensor(f"beta{l}_{d}", [128, NJ], FP, kind="ExternalInput")
        for l in range(3)
        for d in range(2)
    }
    grep_d = {
        (l, d): nc.dram_tensor(f"grep{l}_{d}", [128, 8, 32], FP, kind="ExternalInput")
        for l in range(3)
        for d in range(2)
    }
    fcw_d = nc.dram_tensor("fcw", [128, 16, 60], BF if SCAN_BF16 else FP, kind="ExternalInput")
    fcb_d = nc.dram_tensor("fcb", [60, 1], FP, kind="ExternalInput")

    x1_d = nc.dram_tensor("x1", [128, 16, NT], PDT)
    gi_d = {d: nc.dram_tensor(f"gi_{d}", [128, NJ, NT], FP) for d in range(2)}
    enc_d = nc.dram_tensor("enc", [128, 16, NT], FP, kind="ExternalOutput")
    log_d = nc.dram_tensor("logits", [60, NC_B], FP, kind="ExternalOutput")

    with tile.TileContext(nc) as tc:
        with tc.tile_pool(name="hst", bufs=1) as hpool:
            # betas (all layers/dirs) loaded once
            beta_t = {}
            for l in range(3):
                for d in range(2):
                    bt = hpool.tile([128, NJ], FP, tag=f"beta{l}{d}")
                    nc.sync.dma_start(bt[:], beta_d[(l, d)][:])
                    beta_t[(l, d)] = bt

            xbufs = {0: x1_d, 1: x1_d, 2: enc_d}  # output x of layer l
            hfin = {}
            for l in range(3):
                # ---- projection for both dirs ----
                with (
                    tc.tile_pool(name=f"px{l}", bufs=1) as px,
                    tc.tile_pool(name=f"pw{l}", bufs=3) as pw,
                    tc.tile_pool(name=f"pp{l}", bufs=2, space="PSUM") as pp,
                ):
                    with _rep_loop(tc, f"rep_p{l}"):
                        if l == 0:
                            xr = px.tile([128, 2, NT], PDT, tag="xr", name=f"xr{l}")
                            nc.sync.dma_start(xr[:], xt_d[:])
                            kt = 2
                            wsrc = {d: w0_d[d] for d in range(2)}
                        else:
                            xr = px.tile([128, 16, NT], PDT, tag="xr", name=f"xr{l}")
                            nc.sync.dma_start(xr[:], xbufs[l - 1][:])
                            kt = 16
                            wsrc = {d: wih_d[(l, d)] for d in range(2)}
                        for d in range(2):
                            _emit_proj(nc, tc, pw, pp, xr, kt, wsrc[d],
                                       beta_t[(l, d)], gi_d[d])
                # ---- scans (both dirs interleaved) ----
                with (
                    tc.tile_pool(name=f"swh{l}", bufs=1) as swh,
                    tc.tile_pool(name=f"sw{l}", bufs=2) as sw,
                    tc.tile_pool(name=f"sp{l}", bufs=1, space="PSUM") as sp,
                ):
                    hfin[(l, 0)], hfin[(l, 1)] = _emit_scan_pair(
                        nc, tc, swh, sw, sp,
                        {d: whh_d[(l, d)] for d in range(2)},
                        {d: grep_d[(l, d)] for d in range(2)},
                        gi_d, xbufs[l], hpool, l, last=(l == 2),
                    )

            # ---- FC: logits[60, 32] = fc_w @ h_final + fc_b ----
            with (
                tc.tile_pool(name="fcs", bufs=1) as fcs,
                tc.tile_pool(name="fcp", bufs=1, space="PSUM") as fcpp,
            ):
                fcw = fcs.tile([128, 16, 60], BF if SCAN_BF16 else FP, tag="fcw")
                nc.sync.dma_start(fcw[:], fcw_d[:])
                fcb = fcs.tile([60, 1], FP, tag="fcb")
                nc.sync.dma_start(fcb[:], fcb_d[:])
                fp_ = fcpp.tile([60, NC_B], FP, tag="fcp")
                for d in range(2):
                    hf = hfin[(2, d)]
                    for i in range(NI):
                        k = d * 8 + i
                        nc.tensor.matmul(
                            fp_[:],
                            fcw[:, k, :],
                            hf[:, i, :],
                            start=(k == 0),
                            stop=(k == 15),
                        )
                fo = fcs.tile([60, NC_B], FP, tag="fco")
                nc.scalar.activation(fo[:], fp_[:], AF.Identity, bias=fcb[:])
                nc.sync.dma_start(log_d[:], fo[:])

    _split_multi_waits(nc)
    _CACHE["nc"] = nc
    return nc


def _conv_whh(w_hh):
    arr = w_hh.T.reshape(8, 128, G3).transpose(1, 0, 2)
    if SCAN_W_FP8:
        return (arr * FP8_SCALE).astype(ml_dtypes.float8_e4m3fn)
    return arr.astype(BF_NP if SCAN_BF16 else np.float32)


def _pack_gates_T(w):
    """w: [G3, K] -> DRAM layout [128, K//128, 24, 128]: out[p,k,g,c] = w[g*128+c, k*128+p]."""
    K = w.shape[1]
    kt = (K + 127) // 128
    out = np.zeros((128, kt, NJ, 128), np.float32)
    wt = np.ascontiguousarray(w.T)  # [K, G3]
    for k in range(kt):
        rows = wt[k * 128 : (k + 1) * 128]  # [<=128, G3]
        r = rows.shape[0]
        out[:r, k] = rows.reshape(r, NJ, 128)
    return out


def _prep_inputs(x, bn_gamma, bn_beta, ws):
    """Build the 8 per-core input maps. ws[(l,d)] = (w_ih, w_hh, b_ih, b_hh),
    plus fc_w, fc_b in ws['fc']."""
    x = np.asarray(x, np.float32)
    mean = x.mean(axis=(0, 1))
    var = x.var(axis=(0, 1))
    a = np.asarray(bn_gamma, np.float32) / np.sqrt(var + EPS)
    b = np.asarray(bn_beta, np.float32) - mean * a

    shared = {}
    for d in range(2):
        w_ih, w_hh, b_ih, b_hh = ws[(0, d)]
        w0 = np.asarray(w_ih, np.float32) * a[None, :]       # [3072, 150]
        beta = np.asarray(w_ih, np.float32) @ b + np.asarray(b_ih, np.float32)
        beta[: 2 * H] += np.asarray(b_hh, np.float32)[: 2 * H]
        gamma = np.asarray(b_hh, np.float32)[2 * H :]
        shared[f"w0_{d}"] = _pack_gates_T(w0).astype(BF_NP if PROJ_BF16 else np.float32)
        shared[f"beta0_{d}"] = beta.reshape(NJ, 128).T.copy()
        shared[f"grep0_{d}"] = np.repeat(
            gamma.reshape(8, 128).T[:, :, None], NC_B, axis=2
        ).astype(np.float32)
        shared[f"whh0_{d}"] = _conv_whh(np.asarray(w_hh, np.float32))
    for l in (1, 2):
        for d in range(2):
            w_ih, w_hh, b_ih, b_hh = ws[(l, d)]
            beta = np.asarray(b_ih, np.float32).copy()
            beta[: 2 * H] += np.asarray(b_hh, np.float32)[: 2 * H]
            gamma = np.asarray(b_hh, np.float32)[2 * H :]
            shared[f"wih{l}_{d}"] = _pack_gates_T(
                np.asarray(w_ih, np.float32)
            ).astype(BF_NP if PROJ_BF16 else np.float32)
            shared[f"beta{l}_{d}"] = beta.reshape(NJ, 128).T.copy()
            shared[f"grep{l}_{d}"] = np.repeat(
                gamma.reshape(8, 128).T[:, :, None], NC_B, axis=2
            ).astype(np.float32)
            shared[f"whh{l}_{d}"] = _conv_whh(np.asarray(w_hh, np.float32))
    fc_w, fc_b = ws["fc"]
    shared["fcw"] = (
        np.asarray(fc_w, np.float32).T.reshape(16, 128, 60).transpose(1, 0, 2)
        .astype(BF_NP if SCAN_BF16 else np.float32)
    )
    shared["fcb"] = np.asarray(fc_b, np.float32).reshape(60, 1).copy()

    in_maps = []
    for c in range(N_CORES):
        xs = x[c * NC_B : (c + 1) * NC_B]            # [32, 64, 150]
        xtp = np.zeros((128, 2, NT), np.float32)
        xf = xs.transpose(2, 1, 0).reshape(IN_SIZE, NT)  # [150, t*32+n]
        xtp[:, 0, :] = xf[:128]
        xtp[: IN_SIZE - 128, 1, :] = xf[128:]
        m = dict(shared)
        m["xt"] = xtp.astype(BF_NP) if PROJ_BF16 else xtp
        in_maps.append(m)
    return in_maps


def kernel(x, bn_gamma, bn_beta,
           w_ih_0, w_hh_0, b_ih_0, b_hh_0,
           w_ih_1, w_hh_1, b_ih_1, b_hh_1,
           w_ih_2, w_hh_2, b_ih_2, b_hh_2,
           fc_w, fc_b):
    ws = {}
    for l, (wi, wh, bi, bh) in enumerate(
        [(w_ih_0, w_hh_0, b_ih_0, b_hh_0),
         (w_ih_1, w_hh_1, b_ih_1, b_hh_1),
         (w_ih_2, w_hh_2, b_ih_2, b_hh_2)]
    ):
        for d in range(2):
            ws[(l, d)] = (
                np.asarray(wi)[d], np.asarray(wh)[d],
                np.asarray(bi)[d], np.asarray(bh)[d],
            )
    ws["fc"] = (np.asarray(fc_w), np.asarray(fc_b))

    nc = _build()
    in_maps = _prep_inputs(np.asarray(x), bn_gamma, bn_beta, ws)
    res = run_bass_kernel_spmd(nc, in_maps, list(range(N_CORES)))

    enc_parts = []
    log_parts = []
    for c in range(N_CORES):
        enc = res.results[c]["enc"]        # [128, 16, 2048]
        # enc[p, d*8+c, t*32+n] -> [32, 64, 2048]
        e = enc.reshape(128, 16, T, NC_B).transpose(3, 2, 1, 0).reshape(NC_B, T, 2048)
        enc_parts.append(e)
        log_parts.append(res.results[c]["logits"].T)  # [32, 60]
    encoder_hidden = np.concatenate(enc_parts, axis=0)
    out = np.concatenate(log_parts, axis=0)
    return out, encoder_hidden
